# revision 1
# baseline (speedup 1.0000x reference)
"""BlockSparseLinear on 8 TRN2 NeuronCores: out = x @ W^T + bias.

Harness entry point: kernel(**inputs) takes the FULL inputs
(x (8192,4096) f32, weight (4096,4096) f32, bias (4096,) f32) and
returns the FULL output (8192,4096) f32.

Strategy: 8-way data parallel over batch. Each core computes
out^T[:, b_shard] = W @ x^T[:, b_shard] + bias with the PE contraction
dim (SBUF partitions) = in_features. The host pre-blocks every operand
so each DMA is contiguous per partition:

  xk   (32, 128, 1024)     x^T shard, [k_tile, i_local, b]
  w    (32, 128, 32, 128)  W^T blocked, [j_tile, i_local, k_tile, o_local]
  bias (128, 32)           [o_local, j_tile]
  out  (32, 128, 1024)     out^T blocked, [j_tile, o_local, b]

Per core: the x^T shard (16 MB) stays resident in SBUF; the W slab for
one o-tile streams as two 1 MB halves (3 half-slabs of lookahead);
32x32x2 float32r matmuls (full PE rate, ~1.5e-4 rel err) accumulate in
PSUM; DVE fuses the bias add into the PSUM->SBUF copy. Everything is
float32r end-to-end because the BIR verifier requires fp32r matmul
operands to be produced as fp32r (bit-identical to fp32 on the host).
"""

import numpy as np

import concourse.mybir as mybir
import concourse.tile as tile
from concourse import bacc
from concourse.bass_utils import run_bass_kernel_spmd

NCORES = 8
BATCH, INF, OUTF = 8192, 4096, 4096
B = BATCH // NCORES          # per-core batch (1024)
KT = INF // 128              # 32 k-tiles
JT = OUTF // 128             # 32 o-tiles
NB = B // 512                # moving-operand chunks per o-tile (2)
KQ = 16                      # k-tiles per W half-slab (1 MB chunks)

F32 = mybir.dt.float32
F32R = mybir.dt.float32r

_NC_CACHE = {}


def _build_nc():
    if "nc" in _NC_CACHE:
        return _NC_CACHE["nc"]
    nc = bacc.Bacc("TRN2", target_bir_lowering=False, debug=False,
                   num_devices=NCORES)
    x_d = nc.dram_tensor("xk", [KT, 128, B], F32R, kind="ExternalInput")
    w_d = nc.dram_tensor("w", [JT, 128, KT, 128], F32R, kind="ExternalInput")
    b_d = nc.dram_tensor("bias", [128, JT], F32, kind="ExternalInput")
    o_d = nc.dram_tensor("out", [JT, 128, B], F32, kind="ExternalOutput")

    with tile.TileContext(nc) as tc:
        with (
            tc.tile_pool(name="xpool", bufs=1) as xpool,
            tc.tile_pool(name="wpool", bufs=3) as wpool,
            tc.tile_pool(name="bpool", bufs=1) as bpool,
            tc.tile_pool(name="opool", bufs=2) as opool,
            tc.tile_pool(name="pspool", bufs=8, space="PSUM") as pspool,
        ):
            bias_t = bpool.tile([128, JT], F32, tag="bias", name="bias_t")
            nc.scalar.dma_start(out=bias_t[:], in_=b_d[:])

            # resident x^T shard: 32 tiles of [128, B]
            x_tiles = []
            for k in range(KT):
                xt = xpool.tile([128, B], F32R, tag=f"x{k}", name=f"x{k}")
                nc.sync.dma_start(out=xt[:], in_=x_d[k])
                x_tiles.append(xt)

            for j in range(JT):
                # W slab for o-tile j in 1 MB halves for finer pipelining
                wts = []
                for q in range(KT // KQ):
                    wq = wpool.tile([128, KQ, 128], F32R, tag=f"wq{q}",
                                    name=f"wq{q}")
                    nc.scalar.dma_start(out=wq[:],
                                        in_=w_d[j, :, KQ * q:KQ * (q + 1)])
                    wts.append(wq)

                ps = [pspool.tile([128, 512], F32, tag="ps", name=f"ps{h}")
                      for h in range(NB)]
                for k in range(KT):
                    lhs = wts[k // KQ][:, k % KQ, :]
                    for h in range(NB):
                        rhs = x_tiles[k][:, 512 * h:512 * (h + 1)]
                        nc.tensor.matmul(
                            ps[h][:], lhs, rhs,
                            start=(k == 0), stop=(k == KT - 1),
                        )

                ot = opool.tile([128, B], F32, tag="o", name="ot")
                for h in range(NB):
                    nc.vector.tensor_scalar_add(
                        ot[:, 512 * h:512 * (h + 1)], ps[h][:],
                        bias_t[:, j:j + 1],
                    )
                    nc.sync.dma_start(out=o_d[j, :, 512 * h:512 * (h + 1)],
                                      in_=ot[:, 512 * h:512 * (h + 1)])

    nc.compile()
    _NC_CACHE["nc"] = nc
    return nc


def kernel(x, weight, bias):
    x = np.asarray(x, dtype=np.float32)
    weight = np.asarray(weight, dtype=np.float32)
    bias = np.asarray(bias, dtype=np.float32)

    nc = _build_nc()

    # host-side re-layout (sharding + making every DMA contiguous)
    xr = x.reshape(NCORES, B, KT, 128).transpose(0, 2, 3, 1)
    wr = np.ascontiguousarray(
        weight.reshape(JT, 128, KT, 128).transpose(0, 3, 2, 1))
    br = np.ascontiguousarray(bias.reshape(JT, 128).T)
    in_maps = [
        {"xk": np.ascontiguousarray(xr[c]), "w": wr, "bias": br}
        for c in range(NCORES)
    ]

    res = run_bass_kernel_spmd(nc, in_maps, list(range(NCORES)))

    out = np.empty((BATCH, OUTF), np.float32)
    for c in range(NCORES):
        out[c * B:(c + 1) * B] = res.results[c]["out"].reshape(OUTF, B).T
    return out



# revision 2
# speedup vs baseline: 1.2522x; 1.2522x over previous
"""BlockSparseLinear on 8 TRN2 NeuronCores: out = x @ W^T + bias.

Harness entry point: kernel(**inputs) takes the FULL inputs
(x (8192,4096) f32, weight (4096,4096) f32, bias (4096,) f32) and
returns the FULL output (8192,4096) f32.

Strategy: 8-way data parallel over batch. Each core computes
out^T[:, b_shard] = W @ x^T[:, b_shard] + bias with the PE contraction
dim (SBUF partitions) = in_features. The host pre-blocks every operand
so each DMA is contiguous per partition:

  xk   (32, 128, 1024)     x^T shard, [k_tile, i_local, b]      bf16
  w    (32, 128, 32, 128)  W^T blocked, [j_tile, i_local, k_tile, o_local] bf16
  bias (128, 32)           [o_local, j_tile]                    f32
  out  (32, 128, 1024)     out^T blocked, [j_tile, o_local, b]  f32

Per core: the x^T shard (8 MB bf16) stays resident in SBUF; the W slab
for one o-tile streams as two 512 KB halves (3 half-slabs of
lookahead); 32x32x2 bf16 matmuls accumulate in PSUM; DVE fuses the
bias add into the PSUM->SBUF copy. bf16 operands stream at 1 col/cycle
without the fp32r in-matmul 4-byte weight-load serialization, and the
2e-2 harness tolerance leaves bf16's ~2e-3 error a 9x margin.
"""

import ml_dtypes
import numpy as np

import concourse.mybir as mybir
import concourse.tile as tile
from concourse import bacc
from concourse.bass_utils import run_bass_kernel_spmd

NCORES = 8
BATCH, INF, OUTF = 8192, 4096, 4096
B = BATCH // NCORES          # per-core batch (1024)
KT = INF // 128              # 32 k-tiles
JT = OUTF // 128             # 32 o-tiles
NB = B // 512                # moving-operand chunks per o-tile (2)
KQ = 16                      # k-tiles per W half-slab (512 KB chunks)

F32 = mybir.dt.float32
BF16 = mybir.dt.bfloat16

_NC_CACHE = {}


def _build_nc():
    if "nc" in _NC_CACHE:
        return _NC_CACHE["nc"]
    nc = bacc.Bacc("TRN2", target_bir_lowering=False, debug=False,
                   num_devices=NCORES)
    x_d = nc.dram_tensor("xk", [KT, 128, B], BF16, kind="ExternalInput")
    w_d = nc.dram_tensor("w", [JT, 128, KT, 128], BF16, kind="ExternalInput")
    b_d = nc.dram_tensor("bias", [128, JT], F32, kind="ExternalInput")
    o_d = nc.dram_tensor("out", [JT, 128, B], F32, kind="ExternalOutput")

    with tile.TileContext(nc) as tc:
        with (
            tc.tile_pool(name="xpool", bufs=1) as xpool,
            tc.tile_pool(name="wpool", bufs=3) as wpool,
            tc.tile_pool(name="bpool", bufs=1) as bpool,
            tc.tile_pool(name="opool", bufs=2) as opool,
            tc.tile_pool(name="pspool", bufs=8, space="PSUM") as pspool,
        ):
            bias_t = bpool.tile([128, JT], F32, tag="bias", name="bias_t")
            nc.scalar.dma_start(out=bias_t[:], in_=b_d[:])

            # resident x^T shard: 32 tiles of [128, B]
            x_tiles = []
            for k in range(KT):
                xt = xpool.tile([128, B], BF16, tag=f"x{k}", name=f"x{k}")
                nc.sync.dma_start(out=xt[:], in_=x_d[k])
                x_tiles.append(xt)

            for j in range(JT):
                # W slab for o-tile j in 512 KB halves for finer pipelining
                wts = []
                for q in range(KT // KQ):
                    wq = wpool.tile([128, KQ, 128], BF16, tag=f"wq{q}",
                                    name=f"wq{q}")
                    nc.scalar.dma_start(out=wq[:],
                                        in_=w_d[j, :, KQ * q:KQ * (q + 1)])
                    wts.append(wq)

                ps = [pspool.tile([128, 512], F32, tag="ps", name=f"ps{h}")
                      for h in range(NB)]
                for k in range(KT):
                    lhs = wts[k // KQ][:, k % KQ, :]
                    for h in range(NB):
                        rhs = x_tiles[k][:, 512 * h:512 * (h + 1)]
                        nc.tensor.matmul(
                            ps[h][:], lhs, rhs,
                            start=(k == 0), stop=(k == KT - 1),
                        )

                ot = opool.tile([128, B], F32, tag="o", name="ot")
                for h in range(NB):
                    nc.vector.tensor_scalar_add(
                        ot[:, 512 * h:512 * (h + 1)], ps[h][:],
                        bias_t[:, j:j + 1],
                    )
                    nc.sync.dma_start(out=o_d[j, :, 512 * h:512 * (h + 1)],
                                      in_=ot[:, 512 * h:512 * (h + 1)])

    nc.compile()
    _NC_CACHE["nc"] = nc
    return nc


def kernel(x, weight, bias):
    x = np.asarray(x, dtype=np.float32)
    weight = np.asarray(weight, dtype=np.float32)
    bias = np.asarray(bias, dtype=np.float32)

    nc = _build_nc()

    # host-side re-layout (sharding + making every DMA contiguous)
    xr = x.astype(ml_dtypes.bfloat16).reshape(
        NCORES, B, KT, 128).transpose(0, 2, 3, 1)
    wr = np.ascontiguousarray(
        weight.astype(ml_dtypes.bfloat16)
        .reshape(JT, 128, KT, 128).transpose(0, 3, 2, 1))
    br = np.ascontiguousarray(bias.reshape(JT, 128).T)
    in_maps = [
        {"xk": np.ascontiguousarray(xr[c]), "w": wr, "bias": br}
        for c in range(NCORES)
    ]

    res = run_bass_kernel_spmd(nc, in_maps, list(range(NCORES)))

    out = np.empty((BATCH, OUTF), np.float32)
    for c in range(NCORES):
        out[c * B:(c + 1) * B] = res.results[c]["out"].reshape(OUTF, B).T
    return out
